# revision 30
# baseline (speedup 1.0000x reference)
"""Trainium2 Bass kernel for nn_Attention_50173807952647.

GQA attention block: qkv projections + partial interleaved RoPE + softmax
attention + output projection, fp32 inputs/outputs.

Sharding: 8 cores; core d owns kv-head d and query heads {2d, 2d+1} for all
4 batches (head/tensor parallel per the GQA grouping). Each core computes a
partial output (its heads' contribution through Wo); host sums partials + bias.

v9 (597us baseline -> 398us): ACT-saturating software pipeline.
  - x transposed+bf16 on the host -> one plain DMA per chunk (replaces
    159us of XBAR DMA_TRANSPOSE on the sync queue).
  - Flat (step, pair) emission: scores/exp of step s+1 start before the
    last attnVs of step s (attnV lags scores by LAG=2 pairs), so the ACT
    exp stream never waits for the in-order PE queue to clear a q-block
    boundary. The exp ACTIVATEs (256 x [128,1024], ~285us total) are the
    binding engine; DVE (~296us: Wo staging copies, rope, normalize) runs
    just under it.
  - Projection for the next batch's chunk and the previous step's Wo
    matmuls are emitted as filler units, TWO drawn after each exp (a
    step's ~23 units must clear within its ~12 draw points or the
    leftovers burst-block the PE queue at the boundary).
  - normalize(s) is emitted with attnV(s,7), one pair into step s+1, so
    its DVE chain finishes well before the s+1 Wo fillers need oT.
  - Step 0 interleaves batch-0 chunk projections between attention pairs
    (pair p only needs key tiles of chunk p//2); each proj stream is
    fully drained before the next starts - a partially-drawn proj
    generator pins both psC slots and deadlocks slot recycling.
  - Output stores merged to [128,1024] bf16, summed on host in fp32.
    Prologue weight/table DMAs spread across ACT and gpsimd SWDGE queues.
  - Optional Schraudolph DVE-exp (DVE_EXP_PAIRS, off by default: DVE is
    the busier engine, and the int16(A*s+B)->bf16 bitcast exp costs
    ~2x rel err). Perils recorded: reciprocal_approx_fast silently
    breaks on a base-partition-64 input; Tile WAR tracking only covers
    readers already emitted, so pool-recycling allocs (batch_tiles) must
    be emitted after the previous batch's last reader.
"""

import sys

import numpy as np

HEADS = 16
KV_HEADS = 8
DIM_HEAD = 64
ROT_DIM = 32
SCALE = DIM_HEAD ** -0.5
B, N, DIM = 4, 2048, 1024
N_CORES = 8
T = B * N  # 8192 tokens
CHUNK = 512  # projection chunk (tokens)
QB = 512  # attention query block
FP8_ATTNV = False
# pairs of key tiles whose exp runs on the DVE (Schraudolph) instead of ACT
import os as _os
DVE_EXP_PAIRS = tuple(
    int(x) for x in _os.environ.get("KDVE_PAIRS", "").split(",") if x != "")
# bf16-domain Schraudolph: e ~ bitcast_bf16(int16(A*s + Bc))
SCHRAU_A = 184.66496280361244  # 128/ln2
SCHRAU_B = 16256.0 - 7.4
EBIAS = -2.0 if FP8_ATTNV else 0.0  # common shift: ACT exp(s+EBIAS), DVE matches

_BUILT = {}


def _ensure_path():
    for p in ("/opt/trn_rl_repo",):
        if p not in sys.path:
            sys.path.insert(0, p)


def _to_bf16(a):
    import ml_dtypes
    return np.ascontiguousarray(np.asarray(a, np.float32).astype(ml_dtypes.bfloat16))


def _rope_tables():
    """cos/sin tables [128, N] for the transposed [hd, t] layout.

    Row r (hd index within a core's 128 q-rows): head-local d = r % 64.
    d < ROT_DIM: cos(t * inv_freq[d//2]); sin with rotate-half sign folded
    (-sin on even d, +sin on odd d). Elsewhere cos=1, sin=0 so a single
    full-width mul+add applies RoPE only where it belongs.
    """
    inv_freq = 1.0 / (10000.0 ** (np.arange(0, ROT_DIM, 2, dtype=np.float64) / ROT_DIM))
    t = np.arange(N, dtype=np.float64)
    freqs = t[None, :] * inv_freq[:, None]  # [16, N]
    cos = np.ones((128, N), dtype=np.float64)
    sin = np.zeros((128, N), dtype=np.float64)
    for r in range(128):
        d = r % 64
        if d < ROT_DIM:
            f = freqs[d // 2]
            cos[r] = np.cos(f)
            sin[r] = (-1.0 if d % 2 == 0 else 1.0) * np.sin(f)
    return cos.astype(np.float32), sin.astype(np.float32)


def _build(debug=False):
    if ("nc", debug) in _BUILT:
        return _BUILT[("nc", debug)]
    _ensure_path()
    import concourse.bass as bass  # noqa: F401
    import concourse.mybir as mybir
    import concourse.tile as tile
    from concourse import bacc
    from concourse.masks import make_identity

    dt = mybir.dt
    f32, bf16 = dt.float32, dt.bfloat16
    f8 = dt.float8e4
    edt = f8 if FP8_ATTNV else bf16
    AF = mybir.ActivationFunctionType
    OP = mybir.AluOpType
    PM = mybir.MatmulPerfMode

    nc = bacc.Bacc("TRN2", target_bir_lowering=False, debug=False)

    xt_in = nc.dram_tensor("xt", [DIM, T], bf16, kind="ExternalInput").ap()
    wq_in = nc.dram_tensor("wq", [DIM, 128], bf16, kind="ExternalInput").ap()
    wkv_in = nc.dram_tensor("wkv", [DIM, 128], bf16, kind="ExternalInput").ap()
    wo_in = nc.dram_tensor("wo", [128, DIM], bf16, kind="ExternalInput").ap()
    cos_in = nc.dram_tensor("cos_t", [128, N], bf16, kind="ExternalInput").ap()
    sin_in = nc.dram_tensor("sin_t", [128, N], bf16, kind="ExternalInput").ap()
    out_d = nc.dram_tensor("out", [T, DIM], bf16, kind="ExternalOutput").ap()
    KDBG = bool(_os.environ.get("KDBG"))
    if KDBG:
        dbg_dve = nc.dram_tensor("dbg_dve", [128, 2048], bf16,
                                 kind="ExternalOutput").ap()
        dbg_act = nc.dram_tensor("dbg_act", [128, 2048], bf16,
                                 kind="ExternalOutput").ap()

    NCH = N // CHUNK  # chunks per batch
    NQB = N // QB  # q blocks per batch
    NKT = N // 128  # key tiles per batch
    NPR = NKT // 2  # key tile pairs
    pair_mask = []
    for i in range(16):
        pair_mask += [2 * i + 1, 2 * i]

    with tile.TileContext(nc) as tc:
        with (
            tc.tile_pool(name="const", bufs=1) as constp,
            tc.tile_pool(name="perbatch", bufs=2) as batchp,
            tc.tile_pool(name="xt", bufs=6) as xtp,
            tc.tile_pool(name="rope", bufs=6) as ropep,
            tc.tile_pool(name="sm", bufs=2) as smp,
            tc.tile_pool(name="exp", bufs=4) as expp,
            tc.tile_pool(name="osb", bufs=4) as osbp,
            tc.tile_pool(name="outsb", bufs=3) as outsbp,
            tc.tile_pool(name="psA", bufs=2, space="PSUM") as psA,
            tc.tile_pool(name="psB", bufs=1, space="PSUM") as psB,
            tc.tile_pool(name="psC", bufs=2, space="PSUM") as psC,
        ):
            # Prologue DMAs spread across issue queues so the sync queue
            # starts the x chunk loads immediately: wq/cos/sin via the ACT
            # queue (feeds the first proj+rope), wkv/wo via gpsimd SWDGE.
            ident = constp.tile([128, 128], bf16)
            make_identity(nc, ident[:])
            wq_sb = constp.tile([128, 8 * 128], bf16, tag="wq")
            wkv_sb = constp.tile([128, 8 * 128], bf16, tag="wkv")
            for et in range(8):
                nc.scalar.dma_start(wq_sb[:, et * 128:(et + 1) * 128],
                                    wq_in[et * 128:(et + 1) * 128, :])
                nc.gpsimd.dma_start(wkv_sb[:, et * 128:(et + 1) * 128],
                                    wkv_in[et * 128:(et + 1) * 128, :])
            wo_sb = constp.tile([128, DIM], bf16, tag="wo")
            nc.gpsimd.dma_start(wo_sb[:], wo_in[:])
            ebias = constp.tile([128, 1], f32, tag="ebias")
            nc.vector.memset(ebias[:], EBIAS)
            cos_sb = constp.tile([128, N], bf16, tag="cos")
            sin_sb = constp.tile([128, N], bf16, tag="sin")
            nc.scalar.dma_start(cos_sb[:], cos_in[:])
            nc.scalar.dma_start(sin_sb[:], sin_in[:])

            # x^T view for chunked loads: [et, 128, tok]
            xt3 = xt_in.rearrange("(et p) t -> et p t", p=128)

            def load_xT_chunk(b, c):
                """One [128, 8*CHUNK] tile per chunk: column block et holds
                x^T[et*128:(et+1)*128, chunk]. Single DMA instruction (3D
                src AP) instead of 8 transposing DMAs."""
                xte = xtp.tile([128, 8 * CHUNK], bf16, tag="xTc")
                dst = xte[:].rearrange("p (et c) -> p et c", et=8)
                t0 = b * N + c * CHUNK
                nc.sync.dma_start(
                    dst, xt3[:, :, t0:t0 + CHUNK].rearrange("et p c -> p et c"))
                return xte

            chunk_seq = [(bb, cc) for bb in range(B) for cc in range(NCH)]
            chunk_tiles = {}

            def ensure_chunk(idx):
                if 0 <= idx < len(chunk_seq) and idx not in chunk_tiles:
                    bb, cc = chunk_seq[idx]
                    chunk_tiles[idx] = load_xT_chunk(bb, cc)

            def proj_fill(b, c, tiles, u):
                """Generator: projection matmuls + rope for chunk c of batch
                b, yielded in PE-sized units so attn_core can interleave.
                u = [128, 8*CHUNK] x^T chunk tile."""
                qT, kT, v_sb = tiles
                cs = slice(c * CHUNK, (c + 1) * CHUNK)
                qps = psC.tile([128, 512], f32, tag="ps_small")
                for et in range(8):
                    nc.tensor.matmul(qps[:],
                                     wq_sb[:, et * 128:(et + 1) * 128],
                                     u[:, et * CHUNK:(et + 1) * CHUNK],
                                     start=(et == 0), stop=(et == 7))
                    if et % 2 == 1:
                        yield
                kvps = psC.tile([128, 512], f32, tag="ps_small")
                for et in range(8):
                    nc.tensor.matmul(kvps[:],
                                     wkv_sb[:, et * 128:(et + 1) * 128],
                                     u[:, et * CHUNK:(et + 1) * CHUNK],
                                     start=(et == 0), stop=(et == 7))
                    if et % 2 == 1:
                        yield
                # rope epilogue: q (DVE only)
                shq = ropep.tile([128, CHUNK], f32, tag="rope")
                nc.vector.stream_shuffle(shq[:], qps[:], pair_mask)
                t1q = ropep.tile([128, CHUNK], f32, tag="rope")
                nc.vector.tensor_tensor(t1q[:], qps[:], cos_sb[:, cs], op=OP.mult)
                t2q = ropep.tile([128, CHUNK], f32, tag="rope")
                nc.vector.tensor_tensor(t2q[:], shq[:], sin_sb[:, cs], op=OP.mult)
                nc.vector.tensor_tensor(qT[:, cs], t1q[:], t2q[:], op=OP.add)
                yield
                # rope epilogue: k -> kT rows 0:64 (DVE only)
                shk = ropep.tile([32, CHUNK], f32, tag="rope")
                nc.vector.stream_shuffle(shk[:], kvps[0:32, :], pair_mask)
                t1k = ropep.tile([64, CHUNK], f32, tag="rope")
                nc.vector.tensor_tensor(t1k[:], kvps[0:64, :], cos_sb[0:64, cs],
                                        op=OP.mult)
                t2k = ropep.tile([32, CHUNK], f32, tag="rope")
                nc.vector.tensor_tensor(t2k[:], shk[:], sin_sb[0:32, cs], op=OP.mult)
                nc.vector.tensor_tensor(kT[0:32, cs], t1k[0:32, :], t2k[:], op=OP.add)
                nc.vector.tensor_copy(kT[32:64, cs], t1k[32:64, :])
                # duplicate k^T into partitions 64:128 so the head-odd score
                # matmul can pair with qT[64:128] (matmul needs equal base
                # partitions). Keep on nc.sync: SWDGE (gpsimd) issue of the
                # two SBUF->SBUF copies measured 15% slower end-to-end.
                nc.sync.dma_start(kT[64:128, cs], kT[0:64, cs])
                yield
                # v staging copy (DVE), then PE transposes + pack
                vts = ropep.tile([64, CHUNK], bf16, tag="ropev")
                nc.vector.tensor_copy(vts[:], kvps[64:128, :])
                yield
                vtp = psC.tile([128, 512], bf16, tag="ps_small")
                for st in range(4):
                    nc.tensor.transpose(vtp[:, st * 128: st * 128 + 64],
                                        vts[:, st * 128:(st + 1) * 128],
                                        ident[0:64, 0:64])
                yield
                v3 = v_sb[:].rearrange("p (kt c) -> p kt c", c=80)
                # one merged copy for all 4 kt of the chunk (3D src AP)
                vtp4 = vtp[:].rearrange("p (st c) -> p st c", st=4)
                nc.vector.tensor_copy(v3[:, c * 4:(c + 1) * 4, 0:64],
                                      vtp4[:, :, 0:64])
                yield

            def wo_fill(b, qb, oT):
                """Generator: a q-block's out-projection. Units sized so the
                psC pool (bufs=2) never stalls the PE queue: between units
                the attention stream gives the DVE time to drain the
                staging copy. Two eh-halves share one [128,1024] SBUF tile
                -> one merged DMA store per ts."""
                for ts in range(4):
                    ob = outsbp.tile([128, 1024], bf16, tag="ob")
                    for eh in range(2):
                        po = psC.tile([128, 512], f32, tag="ps_small")
                        nc.tensor.matmul(po[:],
                                         oT[:, ts * 128:(ts + 1) * 128],
                                         wo_sb[:, eh * 512:(eh + 1) * 512],
                                         start=True, stop=True)
                        nc.vector.tensor_copy(
                            ob[:, eh * 512:(eh + 1) * 512], po[:])
                        yield
                    r0 = b * N + qb * QB + ts * 128
                    nc.sync.dma_start(out_d[r0:r0 + 128, :], ob[:])

            def normalize(b, qb, ops_t):
                """DVE/gpsimd normalize chain + oT assembly (no PE work).
                Emitted right after the step's last attnV so psB frees early
                and oT is ready when the NEXT step's wo fillers fire
                mid-attention."""
                den = smp.tile([1, 1024], f32, tag="den")
                nc.vector.tensor_copy(den[:], ops_t[64:65, :])
                ou = smp.tile([64, 1024], f32, tag="ou")
                nc.vector.tensor_copy(ou[:], ops_t[0:64, :])
                rec = smp.tile([1, 1024], f32, tag="rq")
                nc.vector.reciprocal_approx_fast(rec[:], den[:])
                rb = smp.tile([64, 1024], f32, tag="rb")
                nc.gpsimd.partition_broadcast(rb[:], rec[:])
                # oT assembly stays on DVE: gpsimd TTs measured 1.7us each
                # and delayed oT past the wo filler draws, stalling the PE
                # queue mid-attention (-120us end-to-end)
                oT = osbp.tile([128, QB], bf16, tag="o")
                nc.vector.tensor_tensor(oT[0:64, :], ou[:, 0:512],
                                        rb[0:64, 0:512], op=OP.mult)
                o1 = osbp.tile([64, QB], bf16, tag="o")
                nc.vector.tensor_tensor(o1[:], ou[:, 512:1024],
                                        rb[0:64, 512:1024], op=OP.mult)
                nc.sync.dma_start(oT[64:128, :], o1[:])
                return oT

            def batch_tiles(b):
                qT = batchp.tile([128, N], bf16, tag="qT")
                kT = batchp.tile([128, N], bf16, tag="kT")
                v_sb = batchp.tile([128, NKT * 80], edt, tag="v")
                ones = v_sb[:].rearrange(
                    "p (kt c) -> p kt c", c=80)[:, :, 64:65]
                nc.vector.memset(ones, 1.0)
                return (qT, kT, v_sb)

            def chain(*gens):
                for g in gens:
                    yield from g

            # ---- flat cross-boundary pipeline over (step, pair) ----
            # Scores/exp for (s+1, 0) are emitted BEFORE the last attnVs of
            # step s, so the ACT never waits for the in-order PE queue to
            # clear a step boundary. attnV lags scores/exp by one pair.
            steps = [(bb, qq) for bb in range(B) for qq in range(NQB)]
            NS = len(steps)

            step_tiles = {}   # s -> (qT, kT, v_sb)
            step_ops = {}     # s -> ops_t psum tile
            step_e2 = {}      # (s, pr) -> e2 tile
            step_oT = {}      # s -> normalized oT
            fillers = {}      # s -> filler generator

            tiles0 = batch_tiles(0)

            def emit_scores_exp(s, pr):
                b, qb = steps[s]
                qT, kT, v_sb = step_tiles[s]
                qs = slice(qb * QB, (qb + 1) * QB)
                e2 = expp.tile([128, 2, 1024], edt, tag="e")
                filler = fillers.get(s)
                for half in range(2):
                    kt = 2 * pr + half
                    ks = slice(kt * 128, (kt + 1) * 128)
                    sps = psA.tile([128, 1024], f32, tag="ps_big")
                    nc.tensor.matmul(sps[:, 0:512], kT[0:64, ks],
                                     qT[0:64, qs], start=True, stop=True)
                    nc.tensor.matmul(sps[:, 512:1024], kT[64:128, ks],
                                     qT[64:128, qs], start=True, stop=True)
                    if pr in DVE_EXP_PAIRS:
                        # Schraudolph exp on DVE: bf16 bit pattern built as
                        # int16(A*scale*s + B); the softmax ratio cancels
                        # most of the ~2% interpolation error
                        nc.vector.tensor_scalar(
                            e2[:, half, :].bitcast(dt.int16), sps[:],
                            SCHRAU_A * SCALE,
                            SCHRAU_B + SCHRAU_A * EBIAS,
                            op0=OP.mult, op1=OP.add)
                        if KDBG and s == 0 and pr == min(DVE_EXP_PAIRS) \
                                and half == 1:
                            edbg = expp.tile([128, 2048], bf16, tag="edbg")
                            nc.scalar.activation(
                                edbg[:].rearrange("p (a b) -> p a b", a=2)
                                [:, half, :],
                                sps[:], AF.Exp, scale=SCALE, bias=ebias[:])
                            nc.sync.dma_start(
                                dbg_dve[:],
                                e2[:].rearrange("p a b -> p (a b)"))
                            nc.sync.dma_start(dbg_act[:], edbg[:])
                    else:
                        # bias -2 keeps exp below the fp8e4 max (~240) for
                        # up to ~7.5-sigma scores; softmax shift-invariance
                        # cancels it exactly in O/den
                        nc.scalar.activation(e2[:, half, :], sps[:],
                                             AF.Exp, scale=SCALE,
                                             bias=ebias[:])
                    if filler is not None:
                        # two units per exp: a step's filler has ~23 units
                        # but only ~12 draw points (draws start at pair
                        # LAG); single draws left an 11-unit burst at each
                        # boundary that blocked the PE queue ~7us per step
                        next(filler, None)
                        next(filler, None)
                step_e2[(s, pr)] = e2

            def emit_attnv(s, pr):
                b, qb = steps[s]
                _, _, v_sb = step_tiles[s]
                v3 = v_sb[:].rearrange("p (kt c) -> p kt c", c=80)
                if pr == 0:
                    step_ops[s] = psB.tile([65, 1024], f32, tag="ps_o",
                                           name="ops_t")
                ops_t = step_ops[s]
                e2 = step_e2.pop((s, pr))
                last = pr == NPR - 1
                if FP8_ATTNV:
                    for h in range(2):
                        nc.tensor.matmul(ops_t[:, h * 512:(h + 1) * 512],
                                         v3[:, 2 * pr:2 * pr + 2, 0:65],
                                         e2[:, :, h * 512:(h + 1) * 512],
                                         start=(pr == 0), stop=last,
                                         perf_mode=PM.DoubleRow)
                else:
                    for h in range(2):
                        for kt in (2 * pr, 2 * pr + 1):
                            nc.tensor.matmul(
                                ops_t[:, h * 512:(h + 1) * 512],
                                v3[:, kt, 0:65],
                                e2[:, kt - 2 * pr, h * 512:(h + 1) * 512],
                                start=(kt == 0 and pr == 0),
                                stop=(last and kt == 2 * pr + 1))
                if last:
                    b_, qb_ = steps[s]
                    step_oT[s] = normalize(b_, qb_, step_ops.pop(s))

            def wo_when_ready(s_prev):
                """Lazy wo filler: normalize(s_prev) is emitted one pair
                into step s_prev+1, so the first draw or two no-op."""
                while s_prev not in step_oT:
                    yield
                bp, qp = steps[s_prev]
                yield from wo_fill(bp, qp, step_oT.pop(s_prev))

            def setup_step(s):
                """Allocate tiles / build fillers when step s's emission is
                about to begin."""
                b, qb = steps[s]
                # next batch's tiles allocated at (b, 0)
                if qb == 0 and b + 1 < B:
                    bt = batch_tiles(b + 1)
                    for ss in range(s + NQB, s + 2 * NQB):
                        step_tiles[ss] = bt
                fills = []
                b1 = b + 1
                if b1 < B:
                    idx = b1 * NCH + qb
                    # lookahead capped at +1: deeper prefetch wedges the
                    # in-order SP queue behind xTc slot waits (deadlock).
                    ensure_chunk(idx)
                    ensure_chunk(idx + 1)
                    if s > 0:
                        # Step 0 drains this chunk as a pair-5 preblock
                        # instead: a partially-drawn proj filler pins BOTH
                        # psC slots (finished-but-unread qps + live kvps),
                        # and the step-0 preblock projs then deadlock the
                        # DVE/PE queues on slot recycling.
                        fills.append(proj_fill(b1, qb, step_tiles[s + NQB],
                                               chunk_tiles.pop(idx)[:]))
                if s >= 1:
                    fills.append(wo_when_ready(s - 1))
                fillers[s] = chain(*fills) if fills else None

            def drain_filler(s):
                f = fillers.pop(s, None)
                if f is not None:
                    for _ in f:
                        pass

            # prologue: batch-0 chunk 0 projected up front; chunks 1-3
            # interleave with step-0 attention pairs (pair p only needs key
            # tiles of chunk p//2, queries of chunk 0)
            for j in range(4):
                ensure_chunk(j)
            for ss in range(NQB):
                step_tiles[ss] = tiles0
            n_pro = NCH if _os.environ.get("KNO_S0") else 1
            for c in range(n_pro):
                for _ in proj_fill(0, c, tiles0, chunk_tiles.pop(c)[:]):
                    pass

            # attnV lags scores/exp by TWO pairs: at a step boundary the
            # next step's first scores then issue back-to-back with the
            # previous scores (only the psA WAR on exp(s,7h0) gates them),
            # shrinking the per-step ACT gap to ~sem+MM latency.
            LAG = 2
            pend = []  # (s, pr) pairs awaiting attnv, oldest first
            for s in range(NS):
                for pr in range(NPR):
                    if s == 0 and not _os.environ.get("KNO_S0"):
                        # feed the rest of batch 0's projection (and batch
                        # 1 chunk 0) between the early pairs of step 0;
                        # each proj stream fully drained before the next
                        if pr in (1, 2, 3):
                            for _ in proj_fill(0, pr, tiles0,
                                               chunk_tiles.pop(pr)[:]):
                                pass
                        elif pr == 5 and B > 1:
                            for _ in proj_fill(1, 0, step_tiles[NQB],
                                               chunk_tiles.pop(NCH)[:]):
                                pass
                    emit_scores_exp(s, pr)
                    if len(pend) >= LAG:
                        done = pend.pop(0)
                        emit_attnv(*done)
                        if done[1] == NPR - 1:
                            drain_filler(done[0])
                    pend.append((s, pr))
                    if pr == LAG - 1:
                        # setup AFTER emit_attnv(s-1, NPR-1): batch_tiles'
                        # memset must be emitted after the previous batch's
                        # last attnv (Tile WAR tracking only covers readers
                        # already emitted — earlier placement races the
                        # v-slot reuse)
                        setup_step(s)
            for done in pend:
                emit_attnv(*done)
            drain_filler(NS - 1)
            # epilogue: last step's out-projection
            bl, ql = steps[NS - 1]
            for _ in wo_fill(bl, ql, step_oT.pop(NS - 1)):
                pass

    nc.compile()
    _BUILT[("nc", debug)] = nc
    return nc


def _make_in_maps(x, Wq, Wk, Wv, Wo):
    import ml_dtypes
    cos_t, sin_t = _rope_tables()
    cos_bf = _to_bf16(cos_t)
    sin_bf = _to_bf16(sin_t)
    xt_bf = np.ascontiguousarray(
        np.asarray(x, np.float32).reshape(T, DIM).T.astype(ml_dtypes.bfloat16))
    in_maps = []
    for d in range(N_CORES):
        wq_d = _to_bf16(np.asarray(Wq, np.float32)[:, d * 128:(d + 1) * 128])
        wk_d = np.asarray(Wk, np.float32)[:, d * 64:(d + 1) * 64]
        wv_d = np.asarray(Wv, np.float32)[:, d * 64:(d + 1) * 64]
        wkv_d = _to_bf16(np.concatenate([wk_d, wv_d], axis=1))
        wo_d = _to_bf16(np.asarray(Wo, np.float32)[d * 128:(d + 1) * 128, :])
        in_maps.append({
            "xt": xt_bf, "wq": wq_d, "wkv": wkv_d, "wo": wo_d,
            "cos_t": cos_bf, "sin_t": sin_bf,
        })
    return in_maps


def _run(in_maps, trace=False, trace_kwargs=None, debug=False):
    _ensure_path()
    from concourse.bass_utils import run_bass_kernel_spmd
    nc = _build(debug=debug)
    return run_bass_kernel_spmd(nc, in_maps, list(range(N_CORES)), trace=trace,
                                **(trace_kwargs or {}))


def kernel(x, Wq, Wk, Wv, Wo, bo):
    x = np.asarray(x, dtype=np.float32)
    in_maps = _make_in_maps(np.ascontiguousarray(x.reshape(B, N, DIM)),
                            np.asarray(Wq, np.float32), np.asarray(Wk, np.float32),
                            np.asarray(Wv, np.float32), np.asarray(Wo, np.float32))
    res = _run(in_maps)
    acc = np.zeros((T, DIM), dtype=np.float32)
    for d in range(N_CORES):
        acc += np.asarray(res.results[d]["out"], dtype=np.float32)
    acc += np.asarray(bo, np.float32)[None, :]
    return acc.reshape(B, N, DIM)


# revision 36
# speedup vs baseline: 1.0060x; 1.0060x over previous
"""Trainium2 Bass kernel for nn_Attention_50173807952647.

GQA attention block: qkv projections + partial interleaved RoPE + softmax
attention + output projection, fp32 inputs/outputs.

Sharding: 8 cores; core d owns kv-head d and query heads {2d, 2d+1} for all
4 batches (head/tensor parallel per the GQA grouping). Each core computes a
partial output (its heads' contribution through Wo); host sums partials + bias.

v9 (597us baseline -> 398us): ACT-saturating software pipeline.
  - x transposed+bf16 on the host -> one plain DMA per chunk (replaces
    159us of XBAR DMA_TRANSPOSE on the sync queue).
  - Flat (step, pair) emission: scores/exp of step s+1 start before the
    last attnVs of step s (attnV lags scores by LAG=2 pairs), so the ACT
    exp stream never waits for the in-order PE queue to clear a q-block
    boundary. The exp ACTIVATEs (256 x [128,1024], ~285us total) are the
    binding engine; DVE (~296us: Wo staging copies, rope, normalize) runs
    just under it.
  - Projection for the next batch's chunk and the previous step's Wo
    matmuls are emitted as filler units, TWO drawn after each exp (a
    step's ~23 units must clear within its ~12 draw points or the
    leftovers burst-block the PE queue at the boundary).
  - normalize(s) is emitted with attnV(s,7), one pair into step s+1, so
    its DVE chain finishes well before the s+1 Wo fillers need oT.
  - Step 0 interleaves batch-0 chunk projections between attention pairs
    (pair p only needs key tiles of chunk p//2); each proj stream is
    fully drained before the next starts - a partially-drawn proj
    generator pins both psC slots and deadlocks slot recycling.
  - Output stores merged to [128,1024] bf16, summed on host in fp32.
    Prologue weight/table DMAs spread across ACT and gpsimd SWDGE queues.
  - Optional Schraudolph DVE-exp (DVE_EXP_PAIRS, off by default: DVE is
    the busier engine, and the int16(A*s+B)->bf16 bitcast exp costs
    ~2x rel err). Perils recorded: reciprocal_approx_fast silently
    breaks on a base-partition-64 input; Tile WAR tracking only covers
    readers already emitted, so pool-recycling allocs (batch_tiles) must
    be emitted after the previous batch's last reader.
"""

import sys

import numpy as np

HEADS = 16
KV_HEADS = 8
DIM_HEAD = 64
ROT_DIM = 32
SCALE = DIM_HEAD ** -0.5
B, N, DIM = 4, 2048, 1024
N_CORES = 8
T = B * N  # 8192 tokens
CHUNK = 512  # projection chunk (tokens)
QB = 512  # attention query block
FP8_ATTNV = False
# pairs of key tiles whose exp runs on the DVE (Schraudolph) instead of ACT
import os as _os
DVE_EXP_PAIRS = tuple(
    int(x) for x in _os.environ.get("KDVE_PAIRS", "").split(",") if x != "")
# bf16-domain Schraudolph: e ~ bitcast_bf16(int16(A*s + Bc))
SCHRAU_A = 184.66496280361244  # 128/ln2
SCHRAU_B = 16256.0 - 7.4
EBIAS = -2.0 if FP8_ATTNV else 0.0  # common shift: ACT exp(s+EBIAS), DVE matches

_BUILT = {}


def _ensure_path():
    for p in ("/opt/trn_rl_repo",):
        if p not in sys.path:
            sys.path.insert(0, p)


def _to_bf16(a):
    import ml_dtypes
    return np.ascontiguousarray(np.asarray(a, np.float32).astype(ml_dtypes.bfloat16))


def _rope_tables():
    """cos/sin tables [128, N] for the transposed [hd, t] layout.

    Row r (hd index within a core's 128 q-rows): head-local d = r % 64.
    d < ROT_DIM: cos(t * inv_freq[d//2]); sin with rotate-half sign folded
    (-sin on even d, +sin on odd d). Elsewhere cos=1, sin=0 so a single
    full-width mul+add applies RoPE only where it belongs.
    """
    inv_freq = 1.0 / (10000.0 ** (np.arange(0, ROT_DIM, 2, dtype=np.float64) / ROT_DIM))
    t = np.arange(N, dtype=np.float64)
    freqs = t[None, :] * inv_freq[:, None]  # [16, N]
    cos = np.ones((128, N), dtype=np.float64)
    sin = np.zeros((128, N), dtype=np.float64)
    for r in range(128):
        d = r % 64
        if d < ROT_DIM:
            f = freqs[d // 2]
            cos[r] = np.cos(f)
            sin[r] = (-1.0 if d % 2 == 0 else 1.0) * np.sin(f)
    return cos.astype(np.float32), sin.astype(np.float32)


def _build(debug=False):
    if ("nc", debug) in _BUILT:
        return _BUILT[("nc", debug)]
    _ensure_path()
    import concourse.bass as bass  # noqa: F401
    import concourse.mybir as mybir
    import concourse.tile as tile
    from concourse import bacc
    from concourse.masks import make_identity

    dt = mybir.dt
    f32, bf16 = dt.float32, dt.bfloat16
    f8 = dt.float8e4
    edt = f8 if FP8_ATTNV else bf16
    AF = mybir.ActivationFunctionType
    OP = mybir.AluOpType
    PM = mybir.MatmulPerfMode

    nc = bacc.Bacc("TRN2", target_bir_lowering=False, debug=False)

    xt_in = nc.dram_tensor("xt", [DIM, T], bf16, kind="ExternalInput").ap()
    wq_in = nc.dram_tensor("wq", [DIM, 128], bf16, kind="ExternalInput").ap()
    wkv_in = nc.dram_tensor("wkv", [DIM, 128], bf16, kind="ExternalInput").ap()
    wo_in = nc.dram_tensor("wo", [128, DIM], bf16, kind="ExternalInput").ap()
    cos_in = nc.dram_tensor("cos_t", [128, N], bf16, kind="ExternalInput").ap()
    sin_in = nc.dram_tensor("sin_t", [128, N], bf16, kind="ExternalInput").ap()
    out_d = nc.dram_tensor("out", [T, DIM], bf16, kind="ExternalOutput").ap()
    KDBG = bool(_os.environ.get("KDBG"))
    if KDBG:
        dbg_dve = nc.dram_tensor("dbg_dve", [128, 2048], bf16,
                                 kind="ExternalOutput").ap()
        dbg_act = nc.dram_tensor("dbg_act", [128, 2048], bf16,
                                 kind="ExternalOutput").ap()

    NCH = N // CHUNK  # chunks per batch
    NQB = N // QB  # q blocks per batch
    NKT = N // 128  # key tiles per batch
    NPR = NKT // 2  # key tile pairs
    pair_mask = []
    for i in range(16):
        pair_mask += [2 * i + 1, 2 * i]

    with tile.TileContext(nc) as tc:
        with (
            tc.tile_pool(name="const", bufs=1) as constp,
            tc.tile_pool(name="perbatch", bufs=2) as batchp,
            tc.tile_pool(name="xt", bufs=6) as xtp,
            tc.tile_pool(name="rope", bufs=6) as ropep,
            tc.tile_pool(name="sm", bufs=2) as smp,
            tc.tile_pool(name="exp", bufs=4) as expp,
            tc.tile_pool(name="osb", bufs=4) as osbp,
            tc.tile_pool(name="outsb", bufs=3) as outsbp,
            tc.tile_pool(name="psA", bufs=2, space="PSUM") as psA,
            tc.tile_pool(name="psB", bufs=1, space="PSUM") as psB,
            tc.tile_pool(name="psC", bufs=2, space="PSUM") as psC,
        ):
            # Prologue DMAs spread across issue queues so the sync queue
            # starts the x chunk loads immediately: wq/cos/sin via the ACT
            # queue (feeds the first proj+rope), wkv/wo via gpsimd SWDGE.
            ident = constp.tile([128, 128], bf16)
            make_identity(nc, ident[:])
            wq_sb = constp.tile([128, 8 * 128], bf16, tag="wq")
            wkv_sb = constp.tile([128, 8 * 128], bf16, tag="wkv")
            for et in range(8):
                nc.scalar.dma_start(wq_sb[:, et * 128:(et + 1) * 128],
                                    wq_in[et * 128:(et + 1) * 128, :])
                nc.gpsimd.dma_start(wkv_sb[:, et * 128:(et + 1) * 128],
                                    wkv_in[et * 128:(et + 1) * 128, :])
            wo_sb = constp.tile([128, DIM], bf16, tag="wo")
            nc.gpsimd.dma_start(wo_sb[:], wo_in[:])
            ebias = constp.tile([128, 1], f32, tag="ebias")
            nc.vector.memset(ebias[:], EBIAS)
            cos_sb = constp.tile([128, N], bf16, tag="cos")
            sin_sb = constp.tile([128, N], bf16, tag="sin")
            nc.gpsimd.dma_start(cos_sb[:], cos_in[:])
            nc.gpsimd.dma_start(sin_sb[:], sin_in[:])

            # x^T view for chunked loads: [et, 128, tok]
            xt3 = xt_in.rearrange("(et p) t -> et p t", p=128)

            def load_xT_chunk(b, c):
                """One [128, 8*CHUNK] tile per chunk: column block et holds
                x^T[et*128:(et+1)*128, chunk]. Single DMA instruction (3D
                src AP) instead of 8 transposing DMAs."""
                xte = xtp.tile([128, 8 * CHUNK], bf16, tag="xTc")
                dst = xte[:].rearrange("p (et c) -> p et c", et=8)
                t0 = b * N + c * CHUNK
                nc.sync.dma_start(
                    dst, xt3[:, :, t0:t0 + CHUNK].rearrange("et p c -> p et c"))
                return xte

            chunk_seq = [(bb, cc) for bb in range(B) for cc in range(NCH)]
            chunk_tiles = {}

            def ensure_chunk(idx):
                if 0 <= idx < len(chunk_seq) and idx not in chunk_tiles:
                    bb, cc = chunk_seq[idx]
                    chunk_tiles[idx] = load_xT_chunk(bb, cc)

            def proj_fill(b, c, tiles, u):
                """Generator: projection matmuls + rope for chunk c of batch
                b, yielded in PE-sized units so attn_core can interleave.
                u = [128, 8*CHUNK] x^T chunk tile."""
                qT, kT, v_sb = tiles
                cs = slice(c * CHUNK, (c + 1) * CHUNK)
                qps = psC.tile([128, 512], f32, tag="ps_small")
                for et in range(8):
                    nc.tensor.matmul(qps[:],
                                     wq_sb[:, et * 128:(et + 1) * 128],
                                     u[:, et * CHUNK:(et + 1) * CHUNK],
                                     start=(et == 0), stop=(et == 7))
                    if et % 2 == 1:
                        yield
                kvps = psC.tile([128, 512], f32, tag="ps_small")
                for et in range(8):
                    nc.tensor.matmul(kvps[:],
                                     wkv_sb[:, et * 128:(et + 1) * 128],
                                     u[:, et * CHUNK:(et + 1) * CHUNK],
                                     start=(et == 0), stop=(et == 7))
                    if et % 2 == 1:
                        yield
                # rope epilogue: q (DVE only)
                shq = ropep.tile([128, CHUNK], f32, tag="rope")
                nc.vector.stream_shuffle(shq[:], qps[:], pair_mask)
                t1q = ropep.tile([128, CHUNK], f32, tag="rope")
                nc.vector.tensor_tensor(t1q[:], qps[:], cos_sb[:, cs], op=OP.mult)
                t2q = ropep.tile([128, CHUNK], f32, tag="rope")
                nc.vector.tensor_tensor(t2q[:], shq[:], sin_sb[:, cs], op=OP.mult)
                nc.vector.tensor_tensor(qT[:, cs], t1q[:], t2q[:], op=OP.add)
                yield
                # rope epilogue: k -> kT rows 0:64 (DVE only)
                shk = ropep.tile([32, CHUNK], f32, tag="rope")
                nc.vector.stream_shuffle(shk[:], kvps[0:32, :], pair_mask)
                t1k = ropep.tile([64, CHUNK], f32, tag="rope")
                nc.vector.tensor_tensor(t1k[:], kvps[0:64, :], cos_sb[0:64, cs],
                                        op=OP.mult)
                t2k = ropep.tile([32, CHUNK], f32, tag="rope")
                nc.vector.tensor_tensor(t2k[:], shk[:], sin_sb[0:32, cs], op=OP.mult)
                nc.vector.tensor_tensor(kT[0:32, cs], t1k[0:32, :], t2k[:], op=OP.add)
                nc.vector.tensor_copy(kT[32:64, cs], t1k[32:64, :])
                # duplicate k^T into partitions 64:128 so the head-odd score
                # matmul can pair with qT[64:128] (matmul needs equal base
                # partitions). Keep on nc.sync: SWDGE (gpsimd) issue of the
                # two SBUF->SBUF copies measured 15% slower end-to-end.
                nc.sync.dma_start(kT[64:128, cs], kT[0:64, cs])
                yield
                # v staging copy (DVE), then PE transposes + pack
                vts = ropep.tile([64, CHUNK], bf16, tag="ropev")
                nc.vector.tensor_copy(vts[:], kvps[64:128, :])
                yield
                vtp = psC.tile([128, 512], bf16, tag="ps_small")
                for st in range(4):
                    nc.tensor.transpose(vtp[:, st * 128: st * 128 + 64],
                                        vts[:, st * 128:(st + 1) * 128],
                                        ident[0:64, 0:64])
                yield
                v3 = v_sb[:].rearrange("p (kt c) -> p kt c", c=80)
                # one merged copy for all 4 kt of the chunk (3D src AP)
                vtp4 = vtp[:].rearrange("p (st c) -> p st c", st=4)
                nc.vector.tensor_copy(v3[:, c * 4:(c + 1) * 4, 0:64],
                                      vtp4[:, :, 0:64])
                yield

            def wo_fill(b, qb, oT):
                """Generator: a q-block's out-projection. Units sized so the
                psC pool (bufs=2) never stalls the PE queue: between units
                the attention stream gives the DVE time to drain the
                staging copy. Two eh-halves share one [128,1024] SBUF tile
                -> one merged DMA store per ts."""
                for ts in range(4):
                    ob = outsbp.tile([128, 1024], bf16, tag="ob")
                    for eh in range(2):
                        po = psC.tile([128, 512], f32, tag="ps_small")
                        nc.tensor.matmul(po[:],
                                         oT[:, ts * 128:(ts + 1) * 128],
                                         wo_sb[:, eh * 512:(eh + 1) * 512],
                                         start=True, stop=True)
                        nc.vector.tensor_copy(
                            ob[:, eh * 512:(eh + 1) * 512], po[:])
                        yield
                    r0 = b * N + qb * QB + ts * 128
                    nc.sync.dma_start(out_d[r0:r0 + 128, :], ob[:])

            def normalize(b, qb, ops_t):
                """DVE/gpsimd normalize chain + oT assembly (no PE work).
                Emitted right after the step's last attnV so psB frees early
                and oT is ready when the NEXT step's wo fillers fire
                mid-attention."""
                den = smp.tile([1, 1024], f32, tag="den")
                nc.vector.tensor_copy(den[:], ops_t[64:65, :])
                ou = smp.tile([64, 1024], f32, tag="ou")
                nc.vector.tensor_copy(ou[:], ops_t[0:64, :])
                rec = smp.tile([1, 1024], f32, tag="rq")
                nc.vector.reciprocal_approx_fast(rec[:], den[:])
                rb = smp.tile([64, 1024], f32, tag="rb")
                nc.gpsimd.partition_broadcast(rb[:], rec[:])
                # oT assembly stays on DVE: gpsimd TTs measured 1.7us each
                # and delayed oT past the wo filler draws, stalling the PE
                # queue mid-attention (-120us end-to-end)
                oT = osbp.tile([128, QB], bf16, tag="o")
                nc.vector.tensor_tensor(oT[0:64, :], ou[:, 0:512],
                                        rb[0:64, 0:512], op=OP.mult)
                o1 = osbp.tile([64, QB], bf16, tag="o")
                nc.vector.tensor_tensor(o1[:], ou[:, 512:1024],
                                        rb[0:64, 512:1024], op=OP.mult)
                nc.sync.dma_start(oT[64:128, :], o1[:])
                return oT

            def batch_tiles(b):
                qT = batchp.tile([128, N], bf16, tag="qT")
                kT = batchp.tile([128, N], bf16, tag="kT")
                v_sb = batchp.tile([128, NKT * 80], edt, tag="v")
                ones = v_sb[:].rearrange(
                    "p (kt c) -> p kt c", c=80)[:, :, 64:65]
                nc.vector.memset(ones, 1.0)
                return (qT, kT, v_sb)

            def chain(*gens):
                for g in gens:
                    yield from g

            # ---- flat cross-boundary pipeline over (step, pair) ----
            # Scores/exp for (s+1, 0) are emitted BEFORE the last attnVs of
            # step s, so the ACT never waits for the in-order PE queue to
            # clear a step boundary. attnV lags scores/exp by one pair.
            steps = [(bb, qq) for bb in range(B) for qq in range(NQB)]
            NS = len(steps)

            step_tiles = {}   # s -> (qT, kT, v_sb)
            step_ops = {}     # s -> ops_t psum tile
            step_e2 = {}      # (s, pr) -> e2 tile
            step_oT = {}      # s -> normalized oT
            fillers = {}      # s -> filler generator

            tiles0 = batch_tiles(0)

            def emit_scores_exp(s, pr):
                b, qb = steps[s]
                qT, kT, v_sb = step_tiles[s]
                qs = slice(qb * QB, (qb + 1) * QB)
                e2 = expp.tile([128, 2, 1024], edt, tag="e")
                filler = fillers.get(s)
                for half in range(2):
                    kt = 2 * pr + half
                    ks = slice(kt * 128, (kt + 1) * 128)
                    sps = psA.tile([128, 1024], f32, tag="ps_big")
                    nc.tensor.matmul(sps[:, 0:512], kT[0:64, ks],
                                     qT[0:64, qs], start=True, stop=True)
                    nc.tensor.matmul(sps[:, 512:1024], kT[64:128, ks],
                                     qT[64:128, qs], start=True, stop=True)
                    if pr in DVE_EXP_PAIRS:
                        # Schraudolph exp on DVE: bf16 bit pattern built as
                        # int16(A*scale*s + B); the softmax ratio cancels
                        # most of the ~2% interpolation error
                        nc.vector.tensor_scalar(
                            e2[:, half, :].bitcast(dt.int16), sps[:],
                            SCHRAU_A * SCALE,
                            SCHRAU_B + SCHRAU_A * EBIAS,
                            op0=OP.mult, op1=OP.add)
                        if KDBG and s == 0 and pr == min(DVE_EXP_PAIRS) \
                                and half == 1:
                            edbg = expp.tile([128, 2048], bf16, tag="edbg")
                            nc.scalar.activation(
                                edbg[:].rearrange("p (a b) -> p a b", a=2)
                                [:, half, :],
                                sps[:], AF.Exp, scale=SCALE, bias=ebias[:])
                            nc.sync.dma_start(
                                dbg_dve[:],
                                e2[:].rearrange("p a b -> p (a b)"))
                            nc.sync.dma_start(dbg_act[:], edbg[:])
                    else:
                        # bias -2 keeps exp below the fp8e4 max (~240) for
                        # up to ~7.5-sigma scores; softmax shift-invariance
                        # cancels it exactly in O/den
                        nc.scalar.activation(e2[:, half, :], sps[:],
                                             AF.Exp, scale=SCALE,
                                             bias=ebias[:])
                    if filler is not None:
                        # two units per exp: a step's filler has ~23 units
                        # but only ~12 draw points (draws start at pair
                        # LAG); single draws left an 11-unit burst at each
                        # boundary that blocked the PE queue ~7us per step.
                        # Step 0 carries 4 chained proj streams (52 units)
                        # and needs 4 draws/exp to keep key-tile
                        # availability ahead of the pair schedule.
                        for _ in range(4 if s == 0 else 2):
                            next(filler, None)
                step_e2[(s, pr)] = e2

            def emit_attnv(s, pr):
                b, qb = steps[s]
                _, _, v_sb = step_tiles[s]
                v3 = v_sb[:].rearrange("p (kt c) -> p kt c", c=80)
                if pr == 0:
                    step_ops[s] = psB.tile([65, 1024], f32, tag="ps_o",
                                           name="ops_t")
                ops_t = step_ops[s]
                e2 = step_e2.pop((s, pr))
                last = pr == NPR - 1
                if FP8_ATTNV:
                    for h in range(2):
                        nc.tensor.matmul(ops_t[:, h * 512:(h + 1) * 512],
                                         v3[:, 2 * pr:2 * pr + 2, 0:65],
                                         e2[:, :, h * 512:(h + 1) * 512],
                                         start=(pr == 0), stop=last,
                                         perf_mode=PM.DoubleRow)
                else:
                    for h in range(2):
                        for kt in (2 * pr, 2 * pr + 1):
                            nc.tensor.matmul(
                                ops_t[:, h * 512:(h + 1) * 512],
                                v3[:, kt, 0:65],
                                e2[:, kt - 2 * pr, h * 512:(h + 1) * 512],
                                start=(kt == 0 and pr == 0),
                                stop=(last and kt == 2 * pr + 1))
                if last:
                    b_, qb_ = steps[s]
                    step_oT[s] = normalize(b_, qb_, step_ops.pop(s))

            def wo_when_ready(s_prev):
                """Lazy wo filler: normalize(s_prev) is emitted one pair
                into step s_prev+1, so the first draw or two no-op."""
                while s_prev not in step_oT:
                    yield
                bp, qp = steps[s_prev]
                yield from wo_fill(bp, qp, step_oT.pop(s_prev))

            def setup_step(s):
                """Allocate tiles / build fillers when step s's emission is
                about to begin."""
                b, qb = steps[s]
                # next batch's tiles allocated at (b, 0)
                if qb == 0 and b + 1 < B:
                    bt = batch_tiles(b + 1)
                    for ss in range(s + NQB, s + 2 * NQB):
                        step_tiles[ss] = bt
                fills = []
                b1 = b + 1
                if b1 < B:
                    idx = b1 * NCH + qb
                    # lookahead capped at +1: deeper prefetch wedges the
                    # in-order SP queue behind xTc slot waits (deadlock).
                    ensure_chunk(idx)
                    ensure_chunk(idx + 1)
                    if s > 0:
                        # Step 0 drains this chunk as a pair-5 preblock
                        # instead: a partially-drawn proj filler pins BOTH
                        # psC slots (finished-but-unread qps + live kvps),
                        # and the step-0 preblock projs then deadlock the
                        # DVE/PE queues on slot recycling.
                        fills.append(proj_fill(b1, qb, step_tiles[s + NQB],
                                               chunk_tiles.pop(idx)[:]))
                if s >= 1:
                    fills.append(wo_when_ready(s - 1))
                if fills:
                    fillers[s] = chain(*fills)
                elif s not in fillers:  # keep the prologue-built step-0 chain
                    fillers[s] = None

            def drain_filler(s):
                f = fillers.pop(s, None)
                if f is not None:
                    for _ in f:
                        pass

            # prologue: batch-0 chunk 0 projected up front; chunks 1-3 and
            # batch-1 chunk 0 become step-0's chained filler (pair p only
            # needs key tiles of chunk p//2, queries of chunk 0). One
            # chained stream: generators run strictly one-at-a-time, so
            # the psC slot pair is never pinned by two live proj streams.
            for j in range(4):
                ensure_chunk(j)
            for ss in range(NQB):
                step_tiles[ss] = tiles0
            n_pro = NCH if _os.environ.get("KNO_S0") else 1
            for c in range(n_pro):
                for _ in proj_fill(0, c, tiles0, chunk_tiles.pop(c)[:]):
                    pass
            def proj_when_tiles(b1, qb, idx):
                """Lazy: batch b1's tiles appear at setup_step((b1-1)*NQB),
                one pair into the step — no-op draws until then."""
                while b1 * NQB not in step_tiles:
                    yield
                yield from proj_fill(b1, qb, step_tiles[b1 * NQB],
                                     chunk_tiles.pop(idx)[:])

            if not _os.environ.get("KNO_S0"):
                ensure_chunk(NCH)
                ensure_chunk(NCH + 1)
                fillers[0] = chain(
                    proj_fill(0, 1, tiles0, chunk_tiles.pop(1)[:]),
                    proj_fill(0, 2, tiles0, chunk_tiles.pop(2)[:]),
                    proj_fill(0, 3, tiles0, chunk_tiles.pop(3)[:]),
                    proj_when_tiles(1, 0, NCH))

            # attnV lags scores/exp by TWO pairs: at a step boundary the
            # next step's first scores then issue back-to-back with the
            # previous scores (only the psA WAR on exp(s,7h0) gates them),
            # shrinking the per-step ACT gap to ~sem+MM latency.
            LAG = 2
            pend = []  # (s, pr) pairs awaiting attnv, oldest first
            for s in range(NS):
                for pr in range(NPR):
                    emit_scores_exp(s, pr)
                    if len(pend) >= LAG:
                        done = pend.pop(0)
                        emit_attnv(*done)
                        if done[1] == NPR - 1:
                            drain_filler(done[0])
                    pend.append((s, pr))
                    if pr == LAG - 1:
                        # setup AFTER emit_attnv(s-1, NPR-1): batch_tiles'
                        # memset must be emitted after the previous batch's
                        # last attnv (Tile WAR tracking only covers readers
                        # already emitted — earlier placement races the
                        # v-slot reuse)
                        setup_step(s)
            for done in pend:
                emit_attnv(*done)
            drain_filler(NS - 1)
            # epilogue: last step's out-projection
            bl, ql = steps[NS - 1]
            for _ in wo_fill(bl, ql, step_oT.pop(NS - 1)):
                pass

    nc.compile()
    _BUILT[("nc", debug)] = nc
    return nc


def _make_in_maps(x, Wq, Wk, Wv, Wo):
    import ml_dtypes
    cos_t, sin_t = _rope_tables()
    cos_bf = _to_bf16(cos_t)
    sin_bf = _to_bf16(sin_t)
    xt_bf = np.ascontiguousarray(
        np.asarray(x, np.float32).reshape(T, DIM).T.astype(ml_dtypes.bfloat16))
    in_maps = []
    for d in range(N_CORES):
        wq_d = _to_bf16(np.asarray(Wq, np.float32)[:, d * 128:(d + 1) * 128])
        wk_d = np.asarray(Wk, np.float32)[:, d * 64:(d + 1) * 64]
        wv_d = np.asarray(Wv, np.float32)[:, d * 64:(d + 1) * 64]
        wkv_d = _to_bf16(np.concatenate([wk_d, wv_d], axis=1))
        wo_d = _to_bf16(np.asarray(Wo, np.float32)[d * 128:(d + 1) * 128, :])
        in_maps.append({
            "xt": xt_bf, "wq": wq_d, "wkv": wkv_d, "wo": wo_d,
            "cos_t": cos_bf, "sin_t": sin_bf,
        })
    return in_maps


def _run(in_maps, trace=False, trace_kwargs=None, debug=False):
    _ensure_path()
    from concourse.bass_utils import run_bass_kernel_spmd
    nc = _build(debug=debug)
    return run_bass_kernel_spmd(nc, in_maps, list(range(N_CORES)), trace=trace,
                                **(trace_kwargs or {}))


def kernel(x, Wq, Wk, Wv, Wo, bo):
    x = np.asarray(x, dtype=np.float32)
    in_maps = _make_in_maps(np.ascontiguousarray(x.reshape(B, N, DIM)),
                            np.asarray(Wq, np.float32), np.asarray(Wk, np.float32),
                            np.asarray(Wv, np.float32), np.asarray(Wo, np.float32))
    res = _run(in_maps)
    acc = np.zeros((T, DIM), dtype=np.float32)
    for d in range(N_CORES):
        acc += np.asarray(res.results[d]["out"], dtype=np.float32)
    acc += np.asarray(bo, np.float32)[None, :]
    return acc.reshape(B, N, DIM)
